# revision 32
# baseline (speedup 1.0000x reference)
"""Causal self-attention kernel for Trainium2, 8 NeuronCores.

Sharding: core j handles batch j//4 and heads 4*(j%4) .. 4*(j%4)+3
(tensor-parallel over heads within a batch replica group of 4 cores).

Per-core on-device pipeline (all matmuls bf16, fp32 accumulate):
  1. Q^T/K^T = W^T x^T feature-major (c-outer accumulation so the input
     DMA pipelines with the first matmuls); K bias dropped (softmax-
     invariant), Q bias fused into the PSUM->SBUF copy on VectorE.
  2. V computed TOKEN-major directly (x^T-stationary matmuls) - no DMA
     transposes. V bias is folded into b_proj on the host (softmax
     weights sum to 1, so the bias passes through unchanged). Per
     key-block, V is stored as [V_even | ones | V_odd] so each head's
     AV lhsT picks up a shared ones column block that makes the AV
     matmul emit softmax row-sums alongside y.
  3. S^T[k,q] per 128-key block vs 512-query chunks, causal block-
     skipped; the two heads of a pair run as concurrent 64-row PE
     tiles (row groups 0/64). exp on ScalarE (no max subtraction:
     logits are O(0.1) by construction); diagonal blocks masked by a
     triangular multiply (both heads in one strided DVE op).
  4. y^T (+ row-sums) accumulated over key blocks; normalized with
     reciprocal_approx_fast (5x faster than DVE reciprocal).
  5. partial = y^T.T @ W_proj -> [T, C] bf16, DMA'd out.
Host sums the 4 partials per batch and adds the folded bias.
qkv fill for pair 1 and c_proj chunks are interleaved into the
attention emission so the PE never starves while ScalarE chews exp.
"""

import sys

if "/opt/trn_rl_repo" not in sys.path:
    sys.path.insert(0, "/opt/trn_rl_repo")

import numpy as np
import ml_dtypes

B, T, C, H, D = 2, 2048, 1024, 16, 64
SCALE = 0.1 / (D**0.5)
HPC = 4          # heads per core
PAIRS = 2        # head pairs per core (2 heads of 64 feats -> 128 partitions)
NCORES = 8

_CACHE = {}


def build_nc(t=T, reps=1):
    import concourse.mybir as mybir
    import concourse.tile as tile
    from concourse import bacc
    from contextlib import ExitStack

    f32 = mybir.dt.float32
    bf16 = mybir.dt.bfloat16
    Exp = mybir.ActivationFunctionType.Exp

    kblks = t // 128   # 128-wide key blocks per sequence
    qch = t // 512     # 512-wide query chunks per sequence

    nc = bacc.Bacc("TRN2")
    xt = nc.declare_dram_parameter("xt", [C, t], bf16, isOutput=False)
    # wqk columns: [Q0 | K0 | Q1 | K1], 128 each (pair-feature-major)
    wqk = nc.declare_dram_parameter("wqk", [C, 512], bf16, isOutput=False)
    # wv columns: [V_h0 | V_h2 | V_h1 | V_h3] (even heads then odd heads,
    # pair-major within each half)
    wv = nc.declare_dram_parameter("wv", [C, 256], bf16, isOutput=False)
    bq = nc.declare_dram_parameter("bq", [PAIRS, 128, 1], f32, isOutput=False)
    trimask = nc.declare_dram_parameter("trimask", [128, 2, 128], bf16,
                                        isOutput=False)
    wproj = nc.declare_dram_parameter("wproj", [HPC * D, C], bf16,
                                      isOutput=False)
    partial = nc.declare_dram_parameter("partial", [t, C], bf16, isOutput=True)

    with tile.TileContext(nc) as tc, ExitStack() as ctx:
        persist = ctx.enter_context(tc.tile_pool(name="persist", bufs=1))
        psum_s = ctx.enter_context(tc.tile_pool(name="psum_s", bufs=2,
                                                space="PSUM"))
        psum_y = ctx.enter_context(tc.tile_pool(name="psum_y", bufs=3,
                                                space="PSUM"))
        psum_w = ctx.enter_context(tc.tile_pool(name="psum_w", bufs=1,
                                                space="PSUM"))
        pt_pool = ctx.enter_context(tc.tile_pool(name="pt_pool", bufs=6))
        misc = ctx.enter_context(tc.tile_pool(name="misc", bufs=4))

        # ---- persistent SBUF tensors ----
        xt_sb = [persist.tile([128, t], bf16, name=f"xt_sb{c}")
                 for c in range(8)]
        wqk_sb = [persist.tile([128, 512], bf16, name=f"wqk_sb{c}")
                  for c in range(8)]
        wv_sb = [persist.tile([128, 256], bf16, name=f"wv_sb{c}")
                 for c in range(8)]
        bq_sb = [persist.tile([128, 1], f32, name=f"bq_sb{p}")
                 for p in range(PAIRS)]
        mask_sb = persist.tile([128, 2, 128], bf16, name="mask_sb")
        wproj_sb = [persist.tile([128, C], bf16, name=f"wproj_sb{p}")
                    for p in range(PAIRS)]

        QT = [persist.tile([128, t], bf16, name=f"QT{p}") for p in range(PAIRS)]
        KT = [persist.tile([128, t], bf16, name=f"KT{p}") for p in range(PAIRS)]
        yT = [persist.tile([128, t], bf16, name=f"yT{p}") for p in range(PAIRS)]
        # Vtok[kb]: [128 keys, pair, head, 128] with each head's 128 cols =
        # [ones(0:64) | V(64:128)], so every AV matmul emits row-sums on
        # output partitions 0:63 and y on 64:127 (reciprocal_approx_fast
        # only works at base partition 0).
        Vtok = [persist.tile([128, PAIRS, 2, 128], bf16, name=f"Vtok{kb}")
                for kb in range(kblks)]

        # ---- HAM warmup: dependency-free dummy matmuls keep the PE busy
        # through the DMA-paced prologue, so the clock gate stays open
        # (2.4 GHz) instead of oscillating back to 1.2 GHz between the
        # sparse first real matmuls.
        warm_sb = persist.tile([128, 128], bf16, name="warm_sb")
        nc.vector.memset(warm_sb, 1.0)
        warm_ps = psum_w.tile([128, 512], f32, name="warm_ps", tag="w")

        def heartbeat(k):
            for _ in range(k):
                nc.tensor.matmul(warm_ps[:, 0:128], lhsT=warm_sb,
                                 rhs=warm_sb, start=True, stop=True)

        heartbeat(64)

        # ---- DMA loads, ordered by first use ----
        for p in range(PAIRS):
            nc.sync.dma_start(bq_sb[p], bq[p])
        nc.sync.dma_start(mask_sb, trimask[:, :, :])
        for c in range(8):
            nc.sync.dma_start(wqk_sb[c][:, 0:256],
                              wqk[c * 128:(c + 1) * 128, 0:256])
            nc.sync.dma_start(xt_sb[c][:, 0:512],
                              xt[c * 128:(c + 1) * 128, 0:512])
        for c in range(8):
            nc.sync.dma_start(wv_sb[c], wv[c * 128:(c + 1) * 128, :])
            nc.sync.dma_start(xt_sb[c][:, 512:1024],
                              xt[c * 128:(c + 1) * 128, 512:1024])
        for c in range(8):
            nc.sync.dma_start(wqk_sb[c][:, 256:512],
                              wqk[c * 128:(c + 1) * 128, 256:512])
        for c in range(8):
            nc.sync.dma_start(xt_sb[c][:, 1024:1536],
                              xt[c * 128:(c + 1) * 128, 1024:1536])
        for c in range(8):
            nc.sync.dma_start(xt_sb[c][:, 1536:2048],
                              xt[c * 128:(c + 1) * 128, 1536:2048])
        for p in range(PAIRS):
            nc.sync.dma_start(wproj_sb[p], wproj[p * 128:(p + 1) * 128, :])
        for kb in range(kblks):
            nc.gpsimd.memset(Vtok[kb][:, :, :, 0:64], 1.0)

        # ---- qkv Q/K unit: feature f over a 512-token quarter ----
        # f: 0=Q0, 1=K0, 2=Q1, 3=K1 (wqk column blocks)
        def emit_qk_unit(f, tq, hb=0):
            p = f // 2
            is_q = (f % 2) == 0
            dest = QT[p] if is_q else KT[p]
            ps = psum_y.tile([128, 512], f32, name=f"qk_ps{f}_{tq}", tag="y")
            for c in range(8):
                heartbeat(hb)
                nc.tensor.matmul(
                    ps,
                    lhsT=wqk_sb[c][:, f * 128:(f + 1) * 128],
                    rhs=xt_sb[c][:, tq * 512:(tq + 1) * 512],
                    start=(c == 0),
                    stop=(c == 7),
                )
            if is_q:
                nc.vector.tensor_scalar_add(
                    dest[:, tq * 512:(tq + 1) * 512], ps, bq_sb[p])
            else:
                nc.vector.tensor_copy(dest[:, tq * 512:(tq + 1) * 512], ps)

        # ---- V unit: 4 token-blocks, token-major, via x^T-stationary ----
        # tb-outer so each tb's accumulation group finishes before the next
        # start=True clears the shared bank's has_written bits.
        def emit_v_unit(g, hb=0):
            ps = psum_s.tile([128, 4, 2, 2, 64], f32, name=f"v_ps{g}", tag="s")
            for tb in range(4):
                for c in range(8):
                    heartbeat(hb)
                    nc.tensor.matmul(
                        ps[:, tb],
                        lhsT=xt_sb[c][:, (g * 4 + tb) * 128:
                                      (g * 4 + tb + 1) * 128],
                        rhs=wv_sb[c],
                        start=(c == 0),
                        stop=(c == 7),
                    )
            for tb in range(4):
                kb = g * 4 + tb
                # even heads -> half 0, odd heads -> half 1 (cols 64:128)
                nc.vector.tensor_copy(Vtok[kb][:, :, 0, 64:128], ps[:, tb, 0])
                nc.vector.tensor_copy(Vtok[kb][:, :, 1, 64:128], ps[:, tb, 1])

        # ---- attention chunk (pair p, 512-query chunk qc) ----
        # AV matmuls lag the S/exp stream by one key block so the AV's
        # exp-wait never head-of-line-blocks the PE queue.
        def emit_attn_chunk(p, qc, filler=None, hb=0):
            yps = [psum_y.tile([128, 512], f32,
                               name=f"y_ps{p}_{qc}_{h}", tag="y")
                   for h in range(2)]
            last_kb = 4 * qc + 3
            pend = []

            def emit_av(kb, pt, n, off):
                for h in range(2):
                    # lhsT = [ones | V] -> out partitions [sums | y]
                    nc.tensor.matmul(
                        yps[h][:, off:512],
                        lhsT=Vtok[kb][:, p, h, :],
                        rhs=pt[:, h, 0:n],
                        start=(kb == 0),
                        stop=(kb == last_kb),
                    )

            for kb in range(4 * qc + 4):
                off = max(0, (kb - 4 * qc) * 128)
                n = 512 - off
                qlo = qc * 512 + off
                s_ps = psum_s.tile([128, 2, 512], f32,
                                   name=f"s_ps{p}_{qc}_{kb}", tag="s")
                pt = pt_pool.tile([128, 2, 512], bf16,
                                  name=f"pt{p}_{qc}_{kb}", tag="pt")
                for h in range(2):
                    nc.tensor.matmul(
                        s_ps[:, h, 0:n],
                        lhsT=KT[p][h * 64:(h + 1) * 64,
                                   kb * 128:(kb + 1) * 128],
                        rhs=QT[p][h * 64:(h + 1) * 64, qlo:(qc + 1) * 512],
                        start=True,
                        stop=True,
                    )
                if n == 512:
                    nc.scalar.activation(pt[:, :, :], s_ps[:, :, :], Exp)
                else:
                    nc.scalar.activation(pt[:, :, 0:n], s_ps[:, :, 0:n], Exp)
                if kb >= 4 * qc:
                    # zero the strictly-lower-triangle (q < k) entries of the
                    # diagonal 128x128 block, both heads in one strided op
                    nc.vector.tensor_mul(pt[:, :, 0:128], pt[:, :, 0:128],
                                         mask_sb)
                pend.append((kb, pt, n, off))
                heartbeat(hb)
                if filler is not None:
                    filler(kb)
                if len(pend) > 1:
                    emit_av(*pend.pop(0))
            while pend:
                emit_av(*pend.pop(0))
            for h in range(2):
                rb = misc.tile([64, 512], f32, name=f"rb{p}_{qc}_{h}",
                               tag="rb")
                nc.vector.reciprocal_approx_fast(rb, yps[h][0:64, :])
                nc.vector.tensor_mul(
                    yT[p][h * 64:(h + 1) * 64, qc * 512:(qc + 1) * 512],
                    yps[h][64:128, :],
                    rb,
                )

        # ---- c_proj for one 128-token block, one 512-output half ----
        def emit_proj_unit(tb, oc, on_scalar=False):
            ps = psum_y.tile([128, 512], f32, name=f"pr_ps{tb}_{oc}", tag="y")
            for p in range(PAIRS):
                nc.tensor.matmul(
                    ps,
                    lhsT=yT[p][:, tb * 128:(tb + 1) * 128],
                    rhs=wproj_sb[p][:, oc * 512:(oc + 1) * 512],
                    start=(p == 0),
                    stop=(p == PAIRS - 1),
                )
            st = misc.tile([128, 512], bf16, name=f"st{tb}_{oc}", tag="st")
            if on_scalar:
                nc.scalar.copy(st, ps)
            else:
                nc.vector.tensor_copy(st, ps)
            nc.sync.dma_start(
                partial[tb * 128:(tb + 1) * 128, oc * 512:(oc + 1) * 512], st)

        for _rep in range(reps):
            # Filler units (independent PE work) are drip-fed into the
            # attention kb loops so the PE keeps streaming while ScalarE
            # chews through the exp backlog. Chunks are emitted pair-
            # interleaved so c_proj (needs both pairs' yT) starts early.
            work = []          # queue of (emit_fn, args)

            def filler(kb):
                if work:
                    fn, args = work.pop(0)
                    fn(*args)

            # Descending chunk order: the biggest chunks (longest exp
            # streams) run first, while independent qkv/V/fill units still
            # exist to keep the PE busy; the cheapest chunks land at the
            # end where only c_proj is left. Pre-work = exactly what
            # attention (0,3) needs, in DMA arrival order.
            emit_qk_unit(1, 0, hb=5)    # K0 tq0
            emit_v_unit(0, hb=1)
            emit_qk_unit(1, 1, hb=2)    # K0 tq1
            emit_v_unit(1, hb=1)
            emit_qk_unit(1, 2, hb=1)    # K0 tq2
            emit_v_unit(2)
            emit_qk_unit(1, 3)          # K0 tq3
            emit_qk_unit(0, 3)          # Q0 tq3

            work += [(emit_v_unit, (3,)), (emit_qk_unit, (0, 2)),
                     (emit_qk_unit, (3, 0))]
            emit_attn_chunk(0, 3, filler=filler, hb=1)
            work += [(emit_qk_unit, (0, 1)), (emit_qk_unit, (3, 1))]
            emit_attn_chunk(0, 2, filler=filler, hb=1)
            work += [(emit_qk_unit, (3, 2)), (emit_qk_unit, (3, 3)),
                     (emit_qk_unit, (2, 3))]
            emit_attn_chunk(0, 1, filler=filler)
            work += [(emit_qk_unit, (0, 0)), (emit_qk_unit, (2, 2))]
            emit_attn_chunk(1, 3, filler=filler)
            for tb in range(12, 16):
                work += [(emit_proj_unit, (tb, 0)), (emit_proj_unit, (tb, 1))]
            work += [(emit_qk_unit, (2, 1))]
            emit_attn_chunk(0, 0, filler=filler)
            emit_attn_chunk(1, 2, filler=filler)
            for tb in range(8, 12):
                work += [(emit_proj_unit, (tb, 0)), (emit_proj_unit, (tb, 1))]
            emit_attn_chunk(1, 1, filler=filler)
            emit_qk_unit(2, 0)
            for tb in range(4, 8):
                work += [(emit_proj_unit, (tb, 0)), (emit_proj_unit, (tb, 1))]

            def filler2(kb):
                filler(kb)
                filler(kb)

            emit_attn_chunk(1, 0, filler=filler2)
            # tail: alternate the PSUM->SBUF copies between ScalarE (idle
            # by now) and VectorE so they overlap
            for tb in range(0, 4):
                work += [(emit_proj_unit, (tb, 0, True)),
                         (emit_proj_unit, (tb, 1, False))]
            while work:
                filler(0)

    return nc


def make_in_maps(x, w_attn, b_attn, w_proj, t=T):
    """Per-core input dicts (host-side shard + layout prep)."""
    bf = ml_dtypes.bfloat16
    tri = np.triu(np.ones((128, 128), np.float32))
    tri2 = np.stack([tri, tri], axis=1).astype(bf)       # [128, 2, 128]
    in_maps = []
    for j in range(NCORES):
        b = j // 4
        hs = [4 * (j % 4) + i for i in range(HPC)]
        cols = np.concatenate([np.arange(h * D, (h + 1) * D) for h in hs])
        # wqk: [Q0 | K0 | Q1 | K1] (Q scaled)
        wparts = []
        bqparts = []
        for p in range(PAIRS):
            pc = cols[p * 128:(p + 1) * 128]
            wparts += [w_attn[:, pc] * SCALE, w_attn[:, C + pc]]
            bqparts.append((b_attn[pc] * SCALE).astype(np.float32)
                           .reshape(128, 1))
        wqk = np.concatenate(wparts, axis=1).astype(bf)
        bq = np.stack(bqparts, axis=0)
        # wv: [V_h0 | V_h2 | V_h1 | V_h3]
        vcols = [np.arange(2 * C + h * D, 2 * C + (h + 1) * D) for h in hs]
        wv = np.concatenate(
            [w_attn[:, vcols[0]], w_attn[:, vcols[2]],
             w_attn[:, vcols[1]], w_attn[:, vcols[3]]], axis=1).astype(bf)
        wproj_j = w_proj[cols, :].astype(bf)
        xt_j = np.ascontiguousarray(x[b, :t].T).astype(bf)
        in_maps.append({
            "xt": xt_j,
            "wqk": wqk,
            "wv": wv,
            "bq": bq,
            "trimask": tri2,
            "wproj": wproj_j,
        })
    return in_maps


def _build_sharded(nc):
    """jit-compiled SPMD executable over 8 cores (mirrors run_bass_via_pjrt),
    returning (callable, in_names, out_names, out_avals, mesh)."""
    import jax
    from jax.experimental.shard_map import shard_map
    from jax.sharding import Mesh, PartitionSpec
    from concourse import bass2jax, mybir
    import numpy as np

    bass2jax.install_neuronx_cc_hook()
    partition_name = nc.partition_id_tensor.name if nc.partition_id_tensor else None
    in_names, out_names, out_avals, zero_shapes = [], [], [], []
    for alloc in nc.m.functions[0].allocations:
        if not isinstance(alloc, mybir.MemoryLocationSet):
            continue
        name = alloc.memorylocations[0].name
        if alloc.kind == "ExternalInput":
            if name != partition_name:
                in_names.append(name)
        elif alloc.kind == "ExternalOutput":
            out_names.append(name)
            shape = tuple(alloc.tensor_shape)
            dtype = mybir.dt.np(alloc.dtype)
            out_avals.append(jax.core.ShapedArray(shape, dtype))
            zero_shapes.append((shape, dtype))
    n_params = len(in_names)
    all_in_names = list(in_names) + list(out_names)
    if partition_name is not None:
        all_in_names.append(partition_name)

    def _body(*args):
        operands = list(args)
        if partition_name is not None:
            operands.append(bass2jax.partition_id_tensor())
        outs = bass2jax._bass_exec_p.bind(
            *operands,
            out_avals=tuple(out_avals),
            in_names=tuple(all_in_names),
            out_names=tuple(out_names),
            lowering_input_output_aliases=(),
            sim_require_finite=True,
            sim_require_nnan=True,
            nc=nc,
        )
        return tuple(outs)

    devices = jax.devices()[:NCORES]
    mesh = Mesh(np.asarray(devices), ("core",))
    n_outs = len(out_names)
    in_specs = (PartitionSpec("core"),) * (n_params + n_outs)
    out_specs = (PartitionSpec("core"),) * n_outs
    donate = tuple(range(n_params, n_params + n_outs))
    sharded = jax.jit(
        shard_map(_body, mesh=mesh, in_specs=in_specs, out_specs=out_specs,
                  check_rep=False),
        donate_argnums=donate,
        keep_unused=True,
    )
    return sharded, in_names, out_names, out_avals, zero_shapes, mesh


def run_spmd(nc, in_maps, iters=0):
    """Execute the SPMD kernel; optionally time `iters` steady-state
    repetitions with device-resident inputs (donated output chaining).
    Returns (per_core_results, per_iter_ns or None)."""
    import time
    import jax
    from jax.sharding import NamedSharding, PartitionSpec

    sharded, in_names, out_names, out_avals, zero_shapes, mesh = _build_sharded(nc)
    n = len(in_maps)
    concat_in = [
        np.concatenate([np.asarray(in_maps[c][name]) for c in range(n)], axis=0)
        for name in in_names
    ]
    zeros = [np.zeros((n * s[0], *s[1:]), d) for s, d in zero_shapes]
    sh = NamedSharding(mesh, PartitionSpec("core"))
    concat_dev = [jax.device_put(a, sh) for a in concat_in]
    zeros_dev = [jax.device_put(z, sh) for z in zeros]

    outs = sharded(*concat_dev, *zeros_dev)
    jax.block_until_ready(outs)
    results = [
        {name: np.asarray(outs[i]).reshape(n, *out_avals[i].shape)[c]
         for i, name in enumerate(out_names)}
        for c in range(n)
    ]
    per_iter_ns = None
    if iters > 0:
        t0 = time.perf_counter()
        cur = outs
        for _ in range(iters):
            cur = sharded(*concat_dev, *cur)
        jax.block_until_ready(cur)
        t1 = time.perf_counter()
        per_iter_ns = (t1 - t0) / iters * 1e9
    return results, per_iter_ns


def assemble_output(results, b_proj, b_attn=None, w_proj=None):
    """Host-side unshard: sum the 4 tensor-parallel partials per batch.
    The V bias is folded in here: b_eff = b_proj + b_v @ w_proj."""
    b_eff = np.asarray(b_proj, np.float32)
    if b_attn is not None and w_proj is not None:
        b_eff = b_eff + np.asarray(b_attn[2 * C:3 * C], np.float32) @ \
            np.asarray(w_proj, np.float32)
    parts = [results[j]["partial"].astype(np.float32) for j in range(NCORES)]
    out = np.empty((B, T, C), np.float32)
    for b in range(B):
        acc = parts[4 * b]
        for j in range(4 * b + 1, 4 * b + 4):
            acc = acc + parts[j]
        out[b] = acc + b_eff[None, :]
    return out


def kernel(x, w_attn, b_attn, w_proj, b_proj, trace=False):
    x = np.asarray(x, np.float32)
    w_attn = np.asarray(w_attn, np.float32)
    b_attn = np.asarray(b_attn, np.float32)
    w_proj = np.asarray(w_proj, np.float32)
    b_proj = np.asarray(b_proj, np.float32)

    if "nc" not in _CACHE:
        nc = build_nc()
        if not nc.is_finalized():
            nc.finalize()
        _CACHE["nc"] = nc
    nc = _CACHE["nc"]

    in_maps = make_in_maps(x, w_attn, b_attn, w_proj)
    iters = int(trace) and 30
    results, per_iter_ns = run_spmd(nc, in_maps, iters=iters)
    _CACHE["per_iter_ns"] = per_iter_ns
    return assemble_output(results, b_proj, b_attn, w_proj)
